# revision 15
# baseline (speedup 1.0000x reference)
"""Mamba2 layer (Granite-style) on 8 Trainium2 NeuronCores.

Sharding: column/row-parallel over heads. Each core computes a 512-channel
slice (8 heads) of gate/x plus the shared B/C channels, runs conv + the
chunked SSD scan for its heads, and produces an un-normalized partial of the
output projection plus its partial sum-of-squares for the gated RMSNorm.
The host applies the per-token rsqrt (which commutes with the out matmul)
while summing the partials.

Device activation is sact(v) = v*(1+tanh(v/2)) = 2*silu(v); the factors of
2 are folded into host-prepared weights so the math is exact.
"""

import numpy as np

HID = 2048
I = 4096
H = 64
P = 64
N = 128
KCONV = 4
CONV_DIM = I + 2 * N           # 4352
PROJ = I + CONV_DIM + H        # 8512
EPS = 1e-6
DT_MAX = 1000000.0
NCORES = 8


class Cfg:
    def __init__(self, nk=16, nb=2, ntt=4, tt=512, nh=8, hid_out=2048):
        self.NK = nk              # hid k-tiles (contraction = NK*128)
        self.NB = nb              # batches
        self.NTT = ntt            # tok-tiles per batch
        self.TT = tt              # tokens per tok-tile (<=512, mult of 128)
        self.NH = nh              # local heads
        self.HIDOUT = hid_out     # out_proj output dim
        self.CL = nh * P          # local channels (gate/x width)
        self.NCH = tt // 128      # chunks per tok-tile
        self.KH = nk * 128        # hid size
        self.T = nb * ntt * tt    # total tokens
        self.XM = self.CL // 128  # x m-tiles
        self.NS = max(1, hid_out // 512)   # out_proj 512-col slices
        self.SW = min(512, hid_out)
        self.KJ = self.CL // 128  # out_proj contraction tiles
        self.NCHT = nb * ntt * self.NCH    # total chunks


def build_nc(cfg):
    import concourse.bass as bass
    import concourse.mybir as mybir
    import concourse.tile as tile
    from concourse import bacc
    from concourse.masks import make_identity
    from concourse._compat import axon_active

    f32 = mybir.dt.float32
    bf16 = mybir.dt.bfloat16
    AF = mybir.ActivationFunctionType
    OP = mybir.AluOpType

    c = cfg
    nc = bacc.Bacc("TRN2", target_bir_lowering=False, debug=not axon_active())

    # DRAM I/O
    hT = nc.dram_tensor("hT", [c.KH, c.T], bf16, kind="ExternalInput")
    wi = nc.dram_tensor("wi", [c.KH, 2 * c.CL + 2 * N], bf16, kind="ExternalInput")
    wo = nc.dram_tensor("wo", [c.CL, c.HIDOUT], bf16, kind="ExternalInput")
    cw = nc.dram_tensor("cw", [c.CL + 2 * N, KCONV], f32, kind="ExternalInput")
    cbh = nc.dram_tensor("cbh", [c.CL + 2 * N, 1], f32, kind="ExternalInput")
    drow = nc.dram_tensor("drow", [1, c.CL], f32, kind="ExternalInput")
    nwrow = nc.dram_tensor("nwrow", [1, c.CL], f32, kind="ExternalInput")
    cumT = nc.dram_tensor("cumT", [c.T, c.NH], f32, kind="ExternalInput")
    cumR = nc.dram_tensor("cumR", [c.NH, c.T], f32, kind="ExternalInput")
    e8 = nc.dram_tensor("e8", [c.T, c.NH], f32, kind="ExternalInput")
    v8 = nc.dram_tensor("v8", [c.T, c.NH], f32, kind="ExternalInput")
    dtm = nc.dram_tensor("dtm", [c.T, c.NH], f32, kind="ExternalInput")
    wl = nc.dram_tensor("wl", [1, c.NCHT * c.NH], f32, kind="ExternalInput")

    pout = nc.dram_tensor("pout", [c.T, c.HIDOUT], bf16, kind="ExternalOutput")
    dbg = {}
    if getattr(c, "DEBUG", False):
        for nm, shp in [("d_xact", [128, c.TT]), ("d_gact", [128, c.CL]),
                        ("d_xall", [128, c.CL]), ("d_gt", [128, 128]),
                        ("d_expm", [128, c.NH * 128]), ("d_py", [128, c.CL]),
                        ("d_py2", [128, c.CL]), ("d_ye", [128, c.CL]),
                        ("d_hf", [128, c.CL])]:
            dbg[nm] = nc.dram_tensor(nm, shp, mybir.dt.float32, kind="ExternalOutput")
    ssq = nc.dram_tensor("ssq", [c.T, 1], f32, kind="ExternalOutput")
    cstate = nc.dram_tensor("cstate", [c.NB, c.CL + 2 * N, KCONV - 1], f32,
                            kind="ExternalOutput")
    hstate = nc.dram_tensor("hstate", [c.NB, N, c.CL], f32, kind="ExternalOutput")

    CM = c.CL + 2 * N            # conv channels
    NM = CM // 128               # conv m-tiles
    GOFF = 0                     # gate col offset in wi
    XOFF = c.CL                  # x/B/C col offset in wi

    with tile.TileContext(nc) as tc:
        with (
            tc.tile_pool(name="const", bufs=1) as cp,
            tc.tile_pool(name="ht", bufs=2) as htp,
            tc.tile_pool(name="raw", bufs=2) as rawp,
            tc.tile_pool(name="xact", bufs=2) as xap,
            tc.tile_pool(name="gate", bufs=c.NCH + 2) as gp,
            tc.tile_pool(name="work", bufs=2) as wp,
            tc.tile_pool(name="mg", bufs=3) as mgp,
            tc.tile_pool(name="dif", bufs=2) as dfp,
            tc.tile_pool(name="cnv", bufs=3) as cvp,
            tc.tile_pool(name="ynt", bufs=6) as ytp,
            tc.tile_pool(name="st", bufs=1) as stp,
            tc.tile_pool(name="pin", bufs=2, space="PSUM") as pin,
            tc.tile_pool(name="ptr", bufs=2, space="PSUM") as ptr,
            tc.tile_pool(name="psc", bufs=2, space="PSUM") as psc,
            tc.tile_pool(name="pop", bufs=2, space="PSUM") as pop,
        ):
            # ---- constants ----
            wi_sb = cp.tile([128, c.NK * (2 * c.CL + 2 * N)], bf16)
            WIW = 2 * c.CL + 2 * N
            for k in range(c.NK):
                nc.sync.dma_start(wi_sb[:, k * WIW:(k + 1) * WIW],
                                  wi.ap()[k * 128:(k + 1) * 128, :])
            wo_sb = cp.tile([128, c.KJ * c.HIDOUT], bf16)
            for j in range(c.KJ):
                nc.sync.dma_start(wo_sb[:, j * c.HIDOUT:(j + 1) * c.HIDOUT],
                                  wo.ap()[j * 128:(j + 1) * 128, :])
            ident = cp.tile([128, 128], bf16)
            make_identity(nc, ident[:])
            cmask = cp.tile([128, 128], f32)
            nc.gpsimd.memset(cmask[:], 0.0)
            nc.gpsimd.affine_select(out=cmask[:], in_=cmask[:],
                                    compare_op=OP.is_ge, fill=-1e30,
                                    base=0, pattern=[[1, 128]],
                                    channel_multiplier=-1)
            cwt = cp.tile([128, NM * KCONV], f32)
            cbt = cp.tile([128, NM], f32)
            for m in range(NM):
                nc.sync.dma_start(cwt[:, m * KCONV:(m + 1) * KCONV],
                                  cw.ap()[m * 128:(m + 1) * 128, :])
                nc.sync.dma_start(cbt[:, m:m + 1],
                                  cbh.ap()[m * 128:(m + 1) * 128, :])
            dr1 = cp.tile([1, c.CL], f32)
            nw1 = cp.tile([1, c.CL], f32)
            nc.sync.dma_start(dr1[:], drow.ap())
            nc.sync.dma_start(nw1[:], nwrow.ap())
            dExp = cp.tile([128, c.CL], f32)
            nwExp = cp.tile([128, c.CL], f32)
            nc.gpsimd.partition_broadcast(dExp[:], dr1[:])
            nc.gpsimd.partition_broadcast(nwExp[:], nw1[:])
            # host scalar chain: [T, NH] -> [128, NCHT*NH] (col block per chunk)
            cumT_sb = cp.tile([128, c.NCHT * c.NH], f32)
            e8_sb = cp.tile([128, c.NCHT * c.NH], f32)
            v8_sb = cp.tile([128, c.NCHT * c.NH], f32)
            dtm_sb = cp.tile([128, c.NCHT * c.NH], f32)
            for ch in range(c.NCHT):
                sl = slice(ch * c.NH, (ch + 1) * c.NH)
                tr = slice(ch * 128, (ch + 1) * 128)
                nc.sync.dma_start(cumT_sb[:, sl], cumT.ap()[tr, :])
                nc.sync.dma_start(e8_sb[:, sl], e8.ap()[tr, :])
                nc.sync.dma_start(v8_sb[:, sl], v8.ap()[tr, :])
                nc.sync.dma_start(dtm_sb[:, sl], dtm.ap()[tr, :])
            wl_sb = cp.tile([1, c.NCHT * c.NH], f32)
            nc.sync.dma_start(wl_sb[:], wl.ap())

            # ---- persistent state ----
            h_f = stp.tile([128, c.CL], f32)
            h_b = stp.tile([128, c.CL], bf16)

            raw_prev = [None] * NM

            for b in range(c.NB):
                for n in range(c.NTT):
                    g = b * c.NTT + n          # global tok-tile
                    t0 = g * c.TT
                    # ---- load hidden^T block ----
                    ht_n = htp.tile([128, c.NK * c.TT], bf16, tag="ht")
                    for k in range(c.NK):
                        nc.sync.dma_start(
                            ht_n[:, k * c.TT:(k + 1) * c.TT],
                            hT.ap()[k * 128:(k + 1) * 128, t0:t0 + c.TT])

                    # ---- in_proj xBC ([ch, tok] orientation) ----
                    raw_cur = []
                    for m in range(NM):
                        ps = pin.tile([128, c.TT], f32, tag="pin")
                        for k in range(c.NK):
                            nc.tensor.matmul(
                                ps[:],
                                wi_sb[:, k * WIW + XOFF + m * 128:
                                      k * WIW + XOFF + (m + 1) * 128],
                                ht_n[:, k * c.TT:(k + 1) * c.TT],
                                start=(k == 0), stop=(k == c.NK - 1))
                        rw = rawp.tile([128, c.TT + 3], bf16, tag=f"raw{m}")
                        if n == 0:
                            nc.vector.memset(rw[:, 0:3], 0.0)
                        else:
                            nc.vector.tensor_copy(rw[:, 0:3],
                                                  raw_prev[m][:, c.TT:c.TT + 3])
                        nc.scalar.copy(rw[:, 3:c.TT + 3], ps[:])
                        if n == c.NTT - 1:
                            cst = wp.tile([128, KCONV - 1], f32, tag="cst")
                            nc.vector.tensor_copy(cst[:], ps[:, c.TT - 3:c.TT])
                            nc.sync.dma_start(
                                cstate.ap()[b, m * 128:(m + 1) * 128, :], cst[:])
                        raw_cur.append(rw)
                        if m == 0:
                            xact = []
                        # ---- conv + sact ----
                        acc = cvp.tile([128, c.TT], f32, tag="cacc")
                        nc.vector.tensor_scalar(
                            out=acc[:], in0=rw[:, 0:c.TT],
                            scalar1=cwt[:, m * KCONV:m * KCONV + 1],
                            scalar2=cbt[:, m:m + 1],
                            op0=OP.mult, op1=OP.add)
                        for kk in range(1, KCONV):
                            acc2 = cvp.tile([128, c.TT], f32, tag="cacc")
                            nc.vector.scalar_tensor_tensor(
                                out=acc2[:], in0=rw[:, kk:kk + c.TT],
                                scalar=cwt[:, m * KCONV + kk:m * KCONV + kk + 1],
                                in1=acc[:], op0=OP.mult, op1=OP.add)
                            acc = acc2
                        th = cvp.tile([128, c.TT], bf16, tag="cth")
                        nc.scalar.activation(out=th[:], in_=acc[:],
                                             func=AF.Tanh, scale=0.5)
                        xa = xap.tile([128, c.TT], bf16, tag=f"xact{m}")
                        nc.vector.scalar_tensor_tensor(
                            out=xa[:], in0=th[:], scalar=1.0, in1=acc[:],
                            op0=OP.add, op1=OP.mult)
                        xact.append(xa)
                        if dbg and g == 0 and m == 0:
                            dx = wp.tile([128, c.TT], f32, tag="dbgx")
                            nc.vector.tensor_copy(dx[:], xa[:])
                            nc.sync.dma_start(dbg["d_xact"].ap()[:, :], dx[:])
                    raw_prev = raw_cur

                    # ---- in_proj gate ([tok, ch] orientation) + sact ----
                    gts = []
                    for j in range(c.NCH):
                        ps = pin.tile([128, c.CL], f32, tag="pin")
                        for k in range(c.NK):
                            nc.tensor.matmul(
                                ps[:],
                                ht_n[:, k * c.TT + j * 128:k * c.TT + (j + 1) * 128],
                                wi_sb[:, k * WIW + GOFF:k * WIW + GOFF + c.CL],
                                start=(k == 0), stop=(k == c.NK - 1))
                        th = cvp.tile([128, c.CL], bf16, tag="gth")
                        nc.scalar.activation(out=th[:], in_=ps[:],
                                             func=AF.Tanh, scale=0.5)
                        gt = gp.tile([128, c.CL], bf16, tag="gt")
                        nc.vector.scalar_tensor_tensor(
                            out=gt[:], in0=th[:], scalar=1.0, in1=ps[:],
                            op0=OP.add, op1=OP.mult)
                        gts.append(gt)
                        if dbg and g == 0 and j == 0:
                            dgx = wp.tile([128, c.CL], f32, tag="dbgg")
                            nc.vector.tensor_copy(dgx[:], gt[:])
                            nc.sync.dma_start(dbg["d_gact"].ap()[:, :], dgx[:])

                    # ---- scan chunks ----
                    for cc in range(c.NCH):
                        ch = g * c.NCH + cc       # global chunk
                        csl = slice(cc * 128, (cc + 1) * 128)
                        if n == 0 and cc == 0:
                            nc.vector.memset(h_f[:], 0.0)
                            nc.vector.memset(h_b[:], 0.0)
                        # x transpose -> xall [tok, ch]
                        xall = wp.tile([128, c.CL], bf16, tag="xall")
                        for m in range(c.XM):
                            pt = ptr.tile([128, 128], bf16, tag="ptr")
                            nc.tensor.transpose(pt[:], xact[m][:, csl], ident[:])
                            nc.scalar.copy(xall[:, m * 128:(m + 1) * 128], pt[:])
                        Bt = xact[c.XM]
                        Ct = xact[c.XM + 1]
                        # Gt
                        pgt = ptr.tile([128, 128], f32, tag="ptr")
                        nc.tensor.matmul(pgt[:], Bt[:, csl], Ct[:, csl],
                                         start=True, stop=True)
                        # B transposed
                        pbt = ptr.tile([128, 128], bf16, tag="ptr")
                        nc.tensor.transpose(pbt[:], Bt[:, csl], ident[:])
                        bT = wp.tile([128, 128], bf16, tag="bT")
                        nc.scalar.copy(bT[:], pbt[:])
                        if dbg and ch == 0:
                            dxa = wp.tile([128, c.CL], f32, tag="dbg1")
                            nc.vector.tensor_copy(dxa[:], xall[:])
                            nc.sync.dma_start(dbg["d_xall"].ap()[:, :], dxa[:])
                            dgt = wp.tile([128, 128], f32, tag="dbg2")
                            nc.vector.tensor_copy(dgt[:], pgt[:])
                            nc.sync.dma_start(dbg["d_gt"].ap()[:, :], dgt[:])
                        # masks + y_intra
                        hsl = slice(ch * c.NH, (ch + 1) * c.NH)
                        diff = dfp.tile([128, c.NH * 128], f32, tag="diff")
                        for hh in range(c.NH):
                            rb = wp.tile([128, 128], f32, tag="rb")
                            nc.sync.dma_start(
                                rb[:], cumR.ap()[hh:hh + 1,
                                                 t0 + cc * 128:t0 + (cc + 1) * 128
                                                 ].broadcast_to([128, 128]))
                            nc.vector.scalar_tensor_tensor(
                                out=diff[:, hh * 128:(hh + 1) * 128],
                                in0=rb[:],
                                scalar=cumT_sb[:, ch * c.NH + hh:ch * c.NH + hh + 1],
                                in1=cmask[:], op0=OP.subtract, op1=OP.add)
                        expm = wp.tile([128, c.NH * 128], bf16, tag="expm")
                        nc.scalar.activation(out=expm[:], in_=diff[:], func=AF.Exp)
                        if dbg and ch == 0:
                            dem = wp.tile([128, c.NH * 128], f32, tag="dbg3")
                            nc.vector.tensor_copy(dem[:], expm[:])
                            nc.sync.dma_start(dbg["d_expm"].ap()[:, :], dem[:])
                        py = psc.tile([128, c.CL], f32, tag="psc")
                        for hh in range(c.NH):
                            mg = mgp.tile([128, 128], bf16, tag="mg")
                            nc.vector.scalar_tensor_tensor(
                                out=mg[:], in0=expm[:, hh * 128:(hh + 1) * 128],
                                scalar=dtm_sb[:, ch * c.NH + hh:ch * c.NH + hh + 1],
                                in1=pgt[:], op0=OP.mult, op1=OP.mult)
                            nc.tensor.matmul(py[:, hh * P:(hh + 1) * P],
                                             mg[:], xall[:, hh * P:(hh + 1) * P],
                                             start=True, stop=True)
                        # inter-chunk term
                        py2 = psc.tile([128, c.CL], f32, tag="psc")
                        nc.tensor.matmul(py2[:], Ct[:, csl], h_b[:],
                                         start=True, stop=True)
                        if dbg and ch == 0:
                            dpy = wp.tile([128, c.CL], f32, tag="dbg4")
                            nc.vector.tensor_copy(dpy[:], py[:])
                            nc.sync.dma_start(dbg["d_py"].ap()[:, :], dpy[:])
                            dp2 = wp.tile([128, c.CL], f32, tag="dbg5")
                            nc.vector.tensor_copy(dp2[:], py2[:])
                            nc.sync.dma_start(dbg["d_py2"].ap()[:, :], dp2[:])
                        # y assembly
                        e3 = e8_sb[:, hsl].unsqueeze(2).broadcast_to([128, c.NH, P])
                        ya = wp.tile([128, c.CL], f32, tag="ya")
                        nc.vector.tensor_tensor(
                            out=ya[:].rearrange("p (a b) -> p a b", a=c.NH),
                            in0=py2[:].rearrange("p (a b) -> p a b", a=c.NH),
                            in1=e3, op=OP.mult)
                        yb = wp.tile([128, c.CL], f32, tag="yb")
                        nc.vector.tensor_tensor(out=yb[:], in0=ya[:], in1=py[:],
                                                op=OP.add)
                        xd = wp.tile([128, c.CL], f32, tag="xd")
                        nc.vector.tensor_tensor(out=xd[:], in0=xall[:], in1=dExp[:],
                                                op=OP.mult)
                        yd = wp.tile([128, c.CL], f32, tag="yd")
                        nc.vector.tensor_tensor(out=yd[:], in0=yb[:], in1=xd[:],
                                                op=OP.add)
                        ye = wp.tile([128, c.CL], f32, tag="ye")
                        nc.vector.tensor_tensor(out=ye[:], in0=yd[:], in1=gts[cc][:],
                                                op=OP.mult)
                        if dbg and ch == 0:
                            dye = wp.tile([128, c.CL], f32, tag="dbg6")
                            nc.vector.tensor_copy(dye[:], ye[:])
                            nc.sync.dma_start(dbg["d_ye"].ap()[:, :], dye[:])
                        sq = wp.tile([128, c.CL], bf16, tag="sq")
                        sqc = wp.tile([128, 1], f32, tag="sqc")
                        nc.scalar.activation(out=sq[:], in_=ye[:], func=AF.Square,
                                             accum_out=sqc[:])
                        nc.sync.dma_start(ssq.ap()[t0 + cc * 128:
                                                   t0 + (cc + 1) * 128, :], sqc[:])
                        yn = wp.tile([128, c.CL], bf16, tag="yn")
                        nc.vector.tensor_tensor(out=yn[:], in0=ye[:], in1=nwExp[:],
                                                op=OP.mult)
                        # state update
                        v3 = v8_sb[:, hsl].unsqueeze(2).broadcast_to([128, c.NH, P])
                        xw2 = wp.tile([128, c.CL], bf16, tag="xw2")
                        nc.vector.tensor_tensor(
                            out=xw2[:].rearrange("p (a b) -> p a b", a=c.NH),
                            in0=xall[:].rearrange("p (a b) -> p a b", a=c.NH),
                            in1=v3, op=OP.mult)
                        pha = psc.tile([128, c.CL], f32, tag="psc")
                        nc.tensor.matmul(pha[:], bT[:], xw2[:],
                                         start=True, stop=True)
                        wlb = wp.tile([128, c.NH], f32, tag="wlb")
                        nc.gpsimd.partition_broadcast(wlb[:], wl_sb[0:1, ch * c.NH:(ch + 1) * c.NH])
                        w3 = wlb[:].unsqueeze(2).broadcast_to([128, c.NH, P])
                        hm = wp.tile([128, c.CL], f32, tag="hm")
                        nc.vector.tensor_tensor(
                            out=hm[:].rearrange("p (a b) -> p a b", a=c.NH),
                            in0=h_f[:].rearrange("p (a b) -> p a b", a=c.NH),
                            in1=w3, op=OP.mult)
                        nc.vector.tensor_tensor(out=h_f[:], in0=hm[:], in1=pha[:],
                                                op=OP.add)
                        if dbg and ch == 0:
                            dhf = wp.tile([128, c.CL], f32, tag="dbg7")
                            nc.vector.tensor_copy(dhf[:], h_f[:])
                            nc.sync.dma_start(dbg["d_hf"].ap()[:, :], dhf[:])
                        nc.scalar.copy(h_b[:], h_f[:])
                        # out_proj
                        ynTs = []
                        for j in range(c.KJ):
                            pt = ptr.tile([128, 128], bf16, tag="ptr")
                            nc.tensor.transpose(pt[:], yn[:, j * 128:(j + 1) * 128],
                                                ident[:])
                            yt = ytp.tile([128, 128], bf16, tag="ynT")
                            nc.scalar.copy(yt[:], pt[:])
                            ynTs.append(yt)
                        for ns in range(c.NS):
                            po = pop.tile([128, c.SW], f32, tag="pop")
                            for j in range(c.KJ):
                                nc.tensor.matmul(
                                    po[:], ynTs[j][:],
                                    wo_sb[:, j * c.HIDOUT + ns * c.SW:
                                          j * c.HIDOUT + (ns + 1) * c.SW],
                                    start=(j == 0), stop=(j == c.KJ - 1))
                            pb = wp.tile([128, c.SW], bf16, tag="pob")
                            nc.scalar.copy(pb[:], po[:])
                            nc.sync.dma_start(
                                pout.ap()[t0 + cc * 128:t0 + (cc + 1) * 128,
                                          ns * c.SW:(ns + 1) * c.SW], pb[:])
                        if n == c.NTT - 1 and cc == c.NCH - 1:
                            nc.sync.dma_start(hstate.ap()[b, :, :], h_f[:])
                            if b < c.NB - 1:
                                nc.vector.memset(h_f[:], 0.0)
                                nc.vector.memset(h_b[:], 0.0)

    nc.compile()
    return nc


def host_prep(hidden_states, W_in, conv_weight, conv_bias, dt_bias, A_log, D,
              norm_weight, W_out, cfg=None):
    """Build the 8 per-core in_maps (and shared tensors)."""
    import ml_dtypes
    bf16 = ml_dtypes.bfloat16
    c = cfg or Cfg()
    b, s, _ = hidden_states.shape
    hid_flat = np.asarray(hidden_states, np.float32).reshape(b * s, HID)
    hTv = np.ascontiguousarray(hid_flat.T).astype(bf16)

    # dt scalar chain (tiny): dt = hidden @ W_dt^T + bias
    W_dt = np.asarray(W_in[I + CONV_DIM:, :], np.float32)       # [H, HID]
    dt = hid_flat @ W_dt.T + np.asarray(dt_bias, np.float32)    # [T, H]
    dtp = np.logaddexp(0.0, dt)                                 # softplus
    dtp = np.clip(dtp, 0.0, DT_MAX)
    A = -np.exp(np.asarray(A_log, np.float32))                  # [H]
    a = dtp * A                                                 # [T, H]
    a4 = a.reshape(b, s // 128, 128, H)
    cum = np.cumsum(a4, axis=2)                                 # chunk-local
    cumL = cum[:, :, -1:, :]
    e8v = np.exp(cum) * 0.5
    v8v = np.exp(cumL - cum) * dtp.reshape(b, s // 128, 128, H) * 0.25
    wlv = np.exp(cumL[:, :, 0, :]).reshape(b * (s // 128), H)
    cumf = cum.reshape(b * s, H)
    dtmv = dtp * 0.125

    in_maps = []
    for i in range(NCORES):
        h0 = i * c.NH
        ch0 = i * c.CL
        rows = np.concatenate([
            np.arange(ch0, ch0 + c.CL),             # gate
            np.arange(I + ch0, I + ch0 + c.CL),     # x
            np.arange(2 * I, 2 * I + 2 * N),        # B, C
        ])
        wiv = np.ascontiguousarray(
            np.asarray(W_in, np.float32)[rows, :].T).astype(bf16)
        wov = np.ascontiguousarray(
            np.asarray(W_out, np.float32)[:, ch0:ch0 + c.CL].T).astype(bf16)
        crows = np.concatenate([
            np.arange(ch0, ch0 + c.CL),
            np.arange(I, I + 2 * N),
        ])
        cwv = np.ascontiguousarray(np.asarray(conv_weight, np.float32)[crows, :])
        cbv = np.ascontiguousarray(
            np.asarray(conv_bias, np.float32)[crows, None])
        drv = np.repeat(np.asarray(D, np.float32)[h0:h0 + c.NH] * 0.5, P)[None, :]
        nwv = (np.asarray(norm_weight, np.float32)[ch0:ch0 + c.CL] * 0.5)[None, :]
        hsl = slice(h0, h0 + c.NH)
        in_maps.append({
            "hT": hTv,
            "wi": wiv,
            "wo": wov,
            "cw": cwv,
            "cbh": cbv,
            "drow": np.ascontiguousarray(drv),
            "nwrow": np.ascontiguousarray(nwv),
            "cumT": np.ascontiguousarray(cumf[:, hsl]),
            "cumR": np.ascontiguousarray(cumf[:, hsl].T),
            "e8": np.ascontiguousarray(e8v.reshape(b * s, H)[:, hsl]),
            "v8": np.ascontiguousarray(v8v.reshape(b * s, H)[:, hsl]),
            "dtm": np.ascontiguousarray(dtmv[:, hsl]),
            "wl": np.ascontiguousarray(wlv[:, hsl]).reshape(1, -1),
        })
    return in_maps


def host_gather(results, b=2, s=2048, cfg=None):
    c = cfg or Cfg()
    pout = np.zeros((b * s, HID), np.float64)
    ssqs = np.zeros((b * s,), np.float64)
    for r in results:
        pout += r["pout"].astype(np.float64)
        ssqs += r["ssq"][:, 0].astype(np.float64) * 0.25
    var = ssqs / I
    rr = 1.0 / np.sqrt(var + EPS)
    out = (pout * rr[:, None]).astype(np.float32).reshape(b, s, HID)

    cs = np.zeros((b, CONV_DIM, KCONV - 1), np.float32)
    for i, r in enumerate(results):
        cs[:, i * c.CL:(i + 1) * c.CL, :] = r["cstate"][:, :c.CL, :]
    cs[:, I:, :] = results[0]["cstate"][:, c.CL:, :]

    hs = np.zeros((b, H, P, N), np.float32)
    for i, r in enumerate(results):
        # hstate [b, N, CL] -> [b, nh, P, N]
        hv = r["hstate"].reshape(b, N, c.NH, P).transpose(0, 2, 3, 1)
        hs[:, i * c.NH:(i + 1) * c.NH] = hv
    return out, cs, hs


_cache = {}
TRACE = False
LAST = {}


def _install_ntff_hook():
    import sys as _sys
    import types as _types
    if 'antenv.axon_hooks' in _sys.modules:
        return
    try:
        import antenv
        mod = _types.ModuleType('antenv.axon_hooks')
        state = {'hook': None}
        mod.set_axon_ntff_profile_hook = lambda h: state.__setitem__('hook', h)
        mod.get_axon_ntff_profile_hook = lambda: state['hook']
        _sys.modules['antenv.axon_hooks'] = mod
        antenv.axon_hooks = mod
        from trn_agent_boot.trn_boot import _ntff_profile_via_ctypes
        mod.set_axon_ntff_profile_hook(
            _ntff_profile_via_ctypes('/opt/axon/libaxon_pjrt.so'))
    except Exception:
        pass


def kernel(hidden_states, W_in, conv_weight, conv_bias, dt_bias, A_log, D,
           norm_weight, W_out):
    from concourse import bass_utils
    cfg = Cfg()
    if "nc" not in _cache:
        _cache["nc"] = build_nc(cfg)
    nc = _cache["nc"]
    in_maps = host_prep(hidden_states, W_in, conv_weight, conv_bias, dt_bias,
                        A_log, D, norm_weight, W_out, cfg)
    if TRACE:
        _install_ntff_hook()
    res = bass_utils.run_bass_kernel_spmd(nc, in_maps,
                                          core_ids=list(range(NCORES)),
                                          trace=TRACE)
    LAST["exec_time_ns"] = res.exec_time_ns
    LAST["results"] = res
    b, s, _ = hidden_states.shape
    return host_gather(res.results, b, s, cfg)


# revision 16
# speedup vs baseline: 1.1250x; 1.1250x over previous
"""Mamba2 layer (Granite-style) on 8 Trainium2 NeuronCores.

Sharding: column/row-parallel over heads. Each core computes a 512-channel
slice (8 heads) of gate/x plus the shared B/C channels, runs conv + the
chunked SSD scan for its heads, and produces an un-normalized partial of the
output projection plus its partial sum-of-squares for the gated RMSNorm.
The host applies the per-token rsqrt (which commutes with the out matmul)
while summing the partials.

Device activation is sact(v) = v*(1+tanh(v/2)) = 2*silu(v); the factors of
2 are folded into host-prepared weights so the math is exact.
"""

import numpy as np

HID = 2048
I = 4096
H = 64
P = 64
N = 128
KCONV = 4
CONV_DIM = I + 2 * N           # 4352
PROJ = I + CONV_DIM + H        # 8512
EPS = 1e-6
DT_MAX = 1000000.0
NCORES = 8


class Cfg:
    def __init__(self, nk=16, nb=2, ntt=4, tt=512, nh=8, hid_out=2048):
        self.NK = nk              # hid k-tiles (contraction = NK*128)
        self.NB = nb              # batches
        self.NTT = ntt            # tok-tiles per batch
        self.TT = tt              # tokens per tok-tile (<=512, mult of 128)
        self.NH = nh              # local heads
        self.HIDOUT = hid_out     # out_proj output dim
        self.CL = nh * P          # local channels (gate/x width)
        self.NCH = tt // 128      # chunks per tok-tile
        self.KH = nk * 128        # hid size
        self.T = nb * ntt * tt    # total tokens
        self.XM = self.CL // 128  # x m-tiles
        self.NS = max(1, hid_out // 512)   # out_proj 512-col slices
        self.SW = min(512, hid_out)
        self.KJ = self.CL // 128  # out_proj contraction tiles
        self.NCHT = nb * ntt * self.NCH    # total chunks


def build_nc(cfg):
    import concourse.bass as bass
    import concourse.mybir as mybir
    import concourse.tile as tile
    from concourse import bacc
    from concourse.masks import make_identity
    from concourse._compat import axon_active

    f32 = mybir.dt.float32
    bf16 = mybir.dt.bfloat16
    AF = mybir.ActivationFunctionType
    OP = mybir.AluOpType

    c = cfg
    nc = bacc.Bacc("TRN2", target_bir_lowering=False, debug=not axon_active())

    # DRAM I/O
    hT = nc.dram_tensor("hT", [c.KH, c.T], bf16, kind="ExternalInput")
    wi = nc.dram_tensor("wi", [c.KH, 2 * c.CL + 2 * N], bf16, kind="ExternalInput")
    wo = nc.dram_tensor("wo", [c.CL, c.HIDOUT], bf16, kind="ExternalInput")
    cw = nc.dram_tensor("cw", [c.CL + 2 * N, KCONV], f32, kind="ExternalInput")
    cbh = nc.dram_tensor("cbh", [c.CL + 2 * N, 1], f32, kind="ExternalInput")
    drow = nc.dram_tensor("drow", [1, c.CL], f32, kind="ExternalInput")
    nwrow = nc.dram_tensor("nwrow", [1, c.CL], f32, kind="ExternalInput")
    cumT = nc.dram_tensor("cumT", [c.T, c.NH], f32, kind="ExternalInput")
    cumR = nc.dram_tensor("cumR", [c.NH, c.T], f32, kind="ExternalInput")
    e8 = nc.dram_tensor("e8", [c.T, c.NH], f32, kind="ExternalInput")
    v8 = nc.dram_tensor("v8", [c.T, c.NH], f32, kind="ExternalInput")
    dtm = nc.dram_tensor("dtm", [c.T, c.NH], f32, kind="ExternalInput")
    wl = nc.dram_tensor("wl", [1, c.NCHT * c.NH], f32, kind="ExternalInput")

    pout = nc.dram_tensor("pout", [c.T, c.HIDOUT], bf16, kind="ExternalOutput")
    dbg = {}
    if getattr(c, "DEBUG", False):
        for nm, shp in [("d_xact", [128, c.TT]), ("d_gact", [128, c.CL]),
                        ("d_xall", [128, c.CL]), ("d_gt", [128, 128]),
                        ("d_expm", [128, c.NH * 128]), ("d_py", [128, c.CL]),
                        ("d_py2", [128, c.CL]), ("d_ye", [128, c.CL]),
                        ("d_hf", [128, c.CL])]:
            dbg[nm] = nc.dram_tensor(nm, shp, mybir.dt.float32, kind="ExternalOutput")
    ssq = nc.dram_tensor("ssq", [c.T, 1], f32, kind="ExternalOutput")
    cstate = nc.dram_tensor("cstate", [c.NB, c.CL + 2 * N, KCONV - 1], f32,
                            kind="ExternalOutput")
    hstate = nc.dram_tensor("hstate", [c.NB, N, c.CL], f32, kind="ExternalOutput")

    CM = c.CL + 2 * N            # conv channels
    NM = CM // 128               # conv m-tiles
    GOFF = 0                     # gate col offset in wi
    XOFF = c.CL                  # x/B/C col offset in wi

    with tile.TileContext(nc) as tc:
        with (
            tc.tile_pool(name="const", bufs=1) as cp,
            tc.tile_pool(name="ht", bufs=2) as htp,
            tc.tile_pool(name="raw", bufs=2) as rawp,
            tc.tile_pool(name="xact", bufs=2) as xap,
            tc.tile_pool(name="gate", bufs=c.NCH + 2) as gp,
            tc.tile_pool(name="work", bufs=2) as wp,
            tc.tile_pool(name="mg", bufs=5) as mgp,
            tc.tile_pool(name="dif", bufs=2) as dfp,
            tc.tile_pool(name="cnv", bufs=3) as cvp,
            tc.tile_pool(name="ynt", bufs=6) as ytp,
            tc.tile_pool(name="st", bufs=1) as stp,
            tc.tile_pool(name="pin", bufs=2, space="PSUM") as pin,
            tc.tile_pool(name="ptr", bufs=2, space="PSUM") as ptr,
            tc.tile_pool(name="psc", bufs=2, space="PSUM") as psc,
            tc.tile_pool(name="pop", bufs=2, space="PSUM") as pop,
        ):
            # ---- constants ----
            wi_sb = cp.tile([128, c.NK * (2 * c.CL + 2 * N)], bf16)
            WIW = 2 * c.CL + 2 * N
            for k in range(c.NK):
                nc.sync.dma_start(wi_sb[:, k * WIW:(k + 1) * WIW],
                                  wi.ap()[k * 128:(k + 1) * 128, :])
            wo_sb = cp.tile([128, c.KJ * c.HIDOUT], bf16)
            for j in range(c.KJ):
                nc.sync.dma_start(wo_sb[:, j * c.HIDOUT:(j + 1) * c.HIDOUT],
                                  wo.ap()[j * 128:(j + 1) * 128, :])
            ident = cp.tile([128, 128], bf16)
            make_identity(nc, ident[:])
            cmask = cp.tile([128, 128], f32)
            nc.gpsimd.memset(cmask[:], 0.0)
            nc.gpsimd.affine_select(out=cmask[:], in_=cmask[:],
                                    compare_op=OP.is_ge, fill=-1e30,
                                    base=0, pattern=[[1, 128]],
                                    channel_multiplier=-1)
            cwt = cp.tile([128, NM * KCONV], f32)
            cbt = cp.tile([128, NM], f32)
            for m in range(NM):
                nc.sync.dma_start(cwt[:, m * KCONV:(m + 1) * KCONV],
                                  cw.ap()[m * 128:(m + 1) * 128, :])
                nc.sync.dma_start(cbt[:, m:m + 1],
                                  cbh.ap()[m * 128:(m + 1) * 128, :])
            dr1 = cp.tile([1, c.CL], f32)
            nw1 = cp.tile([1, c.CL], f32)
            nc.sync.dma_start(dr1[:], drow.ap())
            nc.sync.dma_start(nw1[:], nwrow.ap())
            dExp = cp.tile([128, c.CL], f32)
            nwExp = cp.tile([128, c.CL], f32)
            nc.gpsimd.partition_broadcast(dExp[:], dr1[:])
            nc.gpsimd.partition_broadcast(nwExp[:], nw1[:])
            # host scalar chain: [T, NH] -> [128, NCHT*NH] (col block per chunk)
            cumT_sb = cp.tile([128, c.NCHT * c.NH], f32)
            e8_sb = cp.tile([128, c.NCHT * c.NH], f32)
            v8_sb = cp.tile([128, c.NCHT * c.NH], f32)
            dtm_sb = cp.tile([128, c.NCHT * c.NH], f32)
            for ch in range(c.NCHT):
                sl = slice(ch * c.NH, (ch + 1) * c.NH)
                tr = slice(ch * 128, (ch + 1) * 128)
                nc.sync.dma_start(cumT_sb[:, sl], cumT.ap()[tr, :])
                nc.sync.dma_start(e8_sb[:, sl], e8.ap()[tr, :])
                nc.sync.dma_start(v8_sb[:, sl], v8.ap()[tr, :])
                nc.sync.dma_start(dtm_sb[:, sl], dtm.ap()[tr, :])
            wl_sb = cp.tile([1, c.NCHT * c.NH], f32)
            nc.sync.dma_start(wl_sb[:], wl.ap())

            # ---- persistent state ----
            h_f = stp.tile([128, c.CL], f32)
            h_b = stp.tile([128, c.CL], bf16)

            raw_prev = [None] * NM

            for b in range(c.NB):
                for n in range(c.NTT):
                    g = b * c.NTT + n          # global tok-tile
                    t0 = g * c.TT
                    # ---- load hidden^T block ----
                    ht_n = htp.tile([128, c.NK * c.TT], bf16, tag="ht")
                    for k in range(c.NK):
                        nc.sync.dma_start(
                            ht_n[:, k * c.TT:(k + 1) * c.TT],
                            hT.ap()[k * 128:(k + 1) * 128, t0:t0 + c.TT])

                    # ---- in_proj xBC ([ch, tok] orientation) ----
                    raw_cur = []
                    for m in range(NM):
                        ps = pin.tile([128, c.TT], f32, tag="pin")
                        for k in range(c.NK):
                            nc.tensor.matmul(
                                ps[:],
                                wi_sb[:, k * WIW + XOFF + m * 128:
                                      k * WIW + XOFF + (m + 1) * 128],
                                ht_n[:, k * c.TT:(k + 1) * c.TT],
                                start=(k == 0), stop=(k == c.NK - 1))
                        rw = rawp.tile([128, c.TT + 3], bf16, tag=f"raw{m}")
                        if n == 0:
                            nc.vector.memset(rw[:, 0:3], 0.0)
                        else:
                            nc.vector.tensor_copy(rw[:, 0:3],
                                                  raw_prev[m][:, c.TT:c.TT + 3])
                        nc.scalar.copy(rw[:, 3:c.TT + 3], ps[:])
                        if n == c.NTT - 1:
                            cst = wp.tile([128, KCONV - 1], f32, tag="cst")
                            nc.vector.tensor_copy(cst[:], ps[:, c.TT - 3:c.TT])
                            nc.sync.dma_start(
                                cstate.ap()[b, m * 128:(m + 1) * 128, :], cst[:])
                        raw_cur.append(rw)
                        if m == 0:
                            xact = []
                        # ---- conv + sact ----
                        acc = cvp.tile([128, c.TT], f32, tag="cacc")
                        nc.vector.tensor_scalar(
                            out=acc[:], in0=rw[:, 0:c.TT],
                            scalar1=cwt[:, m * KCONV:m * KCONV + 1],
                            scalar2=cbt[:, m:m + 1],
                            op0=OP.mult, op1=OP.add)
                        for kk in range(1, KCONV):
                            acc2 = cvp.tile([128, c.TT], f32, tag="cacc")
                            nc.vector.scalar_tensor_tensor(
                                out=acc2[:], in0=rw[:, kk:kk + c.TT],
                                scalar=cwt[:, m * KCONV + kk:m * KCONV + kk + 1],
                                in1=acc[:], op0=OP.mult, op1=OP.add)
                            acc = acc2
                        th = cvp.tile([128, c.TT], bf16, tag="cth")
                        nc.scalar.activation(out=th[:], in_=acc[:],
                                             func=AF.Tanh, scale=0.5)
                        xa = xap.tile([128, c.TT], bf16, tag=f"xact{m}")
                        nc.vector.scalar_tensor_tensor(
                            out=xa[:], in0=th[:], scalar=1.0, in1=acc[:],
                            op0=OP.add, op1=OP.mult)
                        xact.append(xa)
                        if dbg and g == 0 and m == 0:
                            dx = wp.tile([128, c.TT], f32, tag="dbgx")
                            nc.vector.tensor_copy(dx[:], xa[:])
                            nc.sync.dma_start(dbg["d_xact"].ap()[:, :], dx[:])
                    raw_prev = raw_cur

                    # ---- in_proj gate ([tok, ch] orientation) + sact ----
                    gts = []
                    for j in range(c.NCH):
                        ps = pin.tile([128, c.CL], f32, tag="pin")
                        for k in range(c.NK):
                            nc.tensor.matmul(
                                ps[:],
                                ht_n[:, k * c.TT + j * 128:k * c.TT + (j + 1) * 128],
                                wi_sb[:, k * WIW + GOFF:k * WIW + GOFF + c.CL],
                                start=(k == 0), stop=(k == c.NK - 1))
                        th = cvp.tile([128, c.CL], bf16, tag="gth")
                        nc.scalar.activation(out=th[:], in_=ps[:],
                                             func=AF.Tanh, scale=0.5)
                        gt = gp.tile([128, c.CL], bf16, tag="gt")
                        nc.vector.scalar_tensor_tensor(
                            out=gt[:], in0=th[:], scalar=1.0, in1=ps[:],
                            op0=OP.add, op1=OP.mult)
                        gts.append(gt)
                        if dbg and g == 0 and j == 0:
                            dgx = wp.tile([128, c.CL], f32, tag="dbgg")
                            nc.vector.tensor_copy(dgx[:], gt[:])
                            nc.sync.dma_start(dbg["d_gact"].ap()[:, :], dgx[:])

                    # ---- scan chunks ----
                    for cc in range(c.NCH):
                        ch = g * c.NCH + cc       # global chunk
                        csl = slice(cc * 128, (cc + 1) * 128)
                        if n == 0 and cc == 0:
                            nc.vector.memset(h_f[:], 0.0)
                            nc.vector.memset(h_b[:], 0.0)
                        # x transpose -> xall [tok, ch]
                        xall = wp.tile([128, c.CL], bf16, tag="xall")
                        for m in range(c.XM):
                            pt = ptr.tile([128, 128], bf16, tag="ptr")
                            nc.tensor.transpose(pt[:], xact[m][:, csl], ident[:])
                            nc.scalar.copy(xall[:, m * 128:(m + 1) * 128], pt[:])
                        Bt = xact[c.XM]
                        Ct = xact[c.XM + 1]
                        # Gt
                        pgt = ptr.tile([128, 128], f32, tag="ptr")
                        nc.tensor.matmul(pgt[:], Bt[:, csl], Ct[:, csl],
                                         start=True, stop=True)
                        # B transposed
                        pbt = ptr.tile([128, 128], bf16, tag="ptr")
                        nc.tensor.transpose(pbt[:], Bt[:, csl], ident[:])
                        bT = wp.tile([128, 128], bf16, tag="bT")
                        nc.scalar.copy(bT[:], pbt[:])
                        if dbg and ch == 0:
                            dxa = wp.tile([128, c.CL], f32, tag="dbg1")
                            nc.vector.tensor_copy(dxa[:], xall[:])
                            nc.sync.dma_start(dbg["d_xall"].ap()[:, :], dxa[:])
                            dgt = wp.tile([128, 128], f32, tag="dbg2")
                            nc.vector.tensor_copy(dgt[:], pgt[:])
                            nc.sync.dma_start(dbg["d_gt"].ap()[:, :], dgt[:])
                        # masks + y_intra
                        hsl = slice(ch * c.NH, (ch + 1) * c.NH)
                        diff = dfp.tile([128, c.NH * 128], f32, tag="diff")
                        for hh in range(c.NH):
                            rb = wp.tile([128, 128], f32, tag="rb")
                            nc.sync.dma_start(
                                rb[:], cumR.ap()[hh:hh + 1,
                                                 t0 + cc * 128:t0 + (cc + 1) * 128
                                                 ].broadcast_to([128, 128]))
                            nc.vector.scalar_tensor_tensor(
                                out=diff[:, hh * 128:(hh + 1) * 128],
                                in0=rb[:],
                                scalar=cumT_sb[:, ch * c.NH + hh:ch * c.NH + hh + 1],
                                in1=cmask[:], op0=OP.subtract, op1=OP.add)
                        expm = wp.tile([128, c.NH * 128], bf16, tag="expm")
                        nc.scalar.activation(out=expm[:], in_=diff[:], func=AF.Exp)
                        if dbg and ch == 0:
                            dem = wp.tile([128, c.NH * 128], f32, tag="dbg3")
                            nc.vector.tensor_copy(dem[:], expm[:])
                            nc.sync.dma_start(dbg["d_expm"].ap()[:, :], dem[:])
                        py = psc.tile([128, c.CL], f32, tag="psc")
                        for hh in range(c.NH):
                            mg = mgp.tile([128, 128], bf16, tag="mg")
                            nc.vector.scalar_tensor_tensor(
                                out=mg[:], in0=expm[:, hh * 128:(hh + 1) * 128],
                                scalar=dtm_sb[:, ch * c.NH + hh:ch * c.NH + hh + 1],
                                in1=pgt[:], op0=OP.mult, op1=OP.mult)
                            nc.tensor.matmul(py[:, hh * P:(hh + 1) * P],
                                             mg[:], xall[:, hh * P:(hh + 1) * P],
                                             start=True, stop=True)
                        # inter-chunk term
                        py2 = pop.tile([128, c.CL], f32, tag="pop")
                        nc.tensor.matmul(py2[:], Ct[:, csl], h_b[:],
                                         start=True, stop=True)
                        if dbg and ch == 0:
                            dpy = wp.tile([128, c.CL], f32, tag="dbg4")
                            nc.vector.tensor_copy(dpy[:], py[:])
                            nc.sync.dma_start(dbg["d_py"].ap()[:, :], dpy[:])
                            dp2 = wp.tile([128, c.CL], f32, tag="dbg5")
                            nc.vector.tensor_copy(dp2[:], py2[:])
                            nc.sync.dma_start(dbg["d_py2"].ap()[:, :], dp2[:])
                        # y assembly
                        e3 = e8_sb[:, hsl].unsqueeze(2).broadcast_to([128, c.NH, P])
                        ya = wp.tile([128, c.CL], f32, tag="ya")
                        nc.vector.tensor_tensor(
                            out=ya[:].rearrange("p (a b) -> p a b", a=c.NH),
                            in0=py2[:].rearrange("p (a b) -> p a b", a=c.NH),
                            in1=e3, op=OP.mult)
                        yb = wp.tile([128, c.CL], f32, tag="yb")
                        nc.vector.tensor_tensor(out=yb[:], in0=ya[:], in1=py[:],
                                                op=OP.add)
                        xd = wp.tile([128, c.CL], f32, tag="xd")
                        nc.vector.tensor_tensor(out=xd[:], in0=xall[:], in1=dExp[:],
                                                op=OP.mult)
                        yd = wp.tile([128, c.CL], f32, tag="yd")
                        nc.vector.tensor_tensor(out=yd[:], in0=yb[:], in1=xd[:],
                                                op=OP.add)
                        ye = wp.tile([128, c.CL], f32, tag="ye")
                        nc.vector.tensor_tensor(out=ye[:], in0=yd[:], in1=gts[cc][:],
                                                op=OP.mult)
                        if dbg and ch == 0:
                            dye = wp.tile([128, c.CL], f32, tag="dbg6")
                            nc.vector.tensor_copy(dye[:], ye[:])
                            nc.sync.dma_start(dbg["d_ye"].ap()[:, :], dye[:])
                        sq = wp.tile([128, c.CL], bf16, tag="sq")
                        sqc = wp.tile([128, 1], f32, tag="sqc")
                        nc.scalar.activation(out=sq[:], in_=ye[:], func=AF.Square,
                                             accum_out=sqc[:])
                        nc.sync.dma_start(ssq.ap()[t0 + cc * 128:
                                                   t0 + (cc + 1) * 128, :], sqc[:])
                        yn = wp.tile([128, c.CL], bf16, tag="yn")
                        nc.vector.tensor_tensor(out=yn[:], in0=ye[:], in1=nwExp[:],
                                                op=OP.mult)
                        # state update
                        v3 = v8_sb[:, hsl].unsqueeze(2).broadcast_to([128, c.NH, P])
                        xw2 = wp.tile([128, c.CL], bf16, tag="xw2")
                        nc.vector.tensor_tensor(
                            out=xw2[:].rearrange("p (a b) -> p a b", a=c.NH),
                            in0=xall[:].rearrange("p (a b) -> p a b", a=c.NH),
                            in1=v3, op=OP.mult)
                        pha = psc.tile([128, c.CL], f32, tag="psc")
                        nc.tensor.matmul(pha[:], bT[:], xw2[:],
                                         start=True, stop=True)
                        wlb = wp.tile([128, c.NH], f32, tag="wlb")
                        nc.gpsimd.partition_broadcast(wlb[:], wl_sb[0:1, ch * c.NH:(ch + 1) * c.NH])
                        w3 = wlb[:].unsqueeze(2).broadcast_to([128, c.NH, P])
                        hm = wp.tile([128, c.CL], f32, tag="hm")
                        nc.vector.tensor_tensor(
                            out=hm[:].rearrange("p (a b) -> p a b", a=c.NH),
                            in0=h_f[:].rearrange("p (a b) -> p a b", a=c.NH),
                            in1=w3, op=OP.mult)
                        nc.vector.tensor_tensor(out=h_f[:], in0=hm[:], in1=pha[:],
                                                op=OP.add)
                        if dbg and ch == 0:
                            dhf = wp.tile([128, c.CL], f32, tag="dbg7")
                            nc.vector.tensor_copy(dhf[:], h_f[:])
                            nc.sync.dma_start(dbg["d_hf"].ap()[:, :], dhf[:])
                        nc.scalar.copy(h_b[:], h_f[:])
                        # out_proj
                        ynTs = []
                        for j in range(c.KJ):
                            pt = ptr.tile([128, 128], bf16, tag="ptr")
                            nc.tensor.transpose(pt[:], yn[:, j * 128:(j + 1) * 128],
                                                ident[:])
                            yt = ytp.tile([128, 128], bf16, tag="ynT")
                            nc.scalar.copy(yt[:], pt[:])
                            ynTs.append(yt)
                        for ns in range(c.NS):
                            po = pop.tile([128, c.SW], f32, tag="pop")
                            for j in range(c.KJ):
                                nc.tensor.matmul(
                                    po[:], ynTs[j][:],
                                    wo_sb[:, j * c.HIDOUT + ns * c.SW:
                                          j * c.HIDOUT + (ns + 1) * c.SW],
                                    start=(j == 0), stop=(j == c.KJ - 1))
                            pb = wp.tile([128, c.SW], bf16, tag="pob")
                            nc.scalar.copy(pb[:], po[:])
                            nc.sync.dma_start(
                                pout.ap()[t0 + cc * 128:t0 + (cc + 1) * 128,
                                          ns * c.SW:(ns + 1) * c.SW], pb[:])
                        if n == c.NTT - 1 and cc == c.NCH - 1:
                            nc.sync.dma_start(hstate.ap()[b, :, :], h_f[:])
                            if b < c.NB - 1:
                                nc.vector.memset(h_f[:], 0.0)
                                nc.vector.memset(h_b[:], 0.0)

    nc.compile()
    return nc


def host_prep(hidden_states, W_in, conv_weight, conv_bias, dt_bias, A_log, D,
              norm_weight, W_out, cfg=None):
    """Build the 8 per-core in_maps (and shared tensors)."""
    import ml_dtypes
    bf16 = ml_dtypes.bfloat16
    c = cfg or Cfg()
    b, s, _ = hidden_states.shape
    hid_flat = np.asarray(hidden_states, np.float32).reshape(b * s, HID)
    hTv = np.ascontiguousarray(hid_flat.T).astype(bf16)

    # dt scalar chain (tiny): dt = hidden @ W_dt^T + bias
    W_dt = np.asarray(W_in[I + CONV_DIM:, :], np.float32)       # [H, HID]
    dt = hid_flat @ W_dt.T + np.asarray(dt_bias, np.float32)    # [T, H]
    dtp = np.logaddexp(0.0, dt)                                 # softplus
    dtp = np.clip(dtp, 0.0, DT_MAX)
    A = -np.exp(np.asarray(A_log, np.float32))                  # [H]
    a = dtp * A                                                 # [T, H]
    a4 = a.reshape(b, s // 128, 128, H)
    cum = np.cumsum(a4, axis=2)                                 # chunk-local
    cumL = cum[:, :, -1:, :]
    e8v = np.exp(cum) * 0.5
    v8v = np.exp(cumL - cum) * dtp.reshape(b, s // 128, 128, H) * 0.25
    wlv = np.exp(cumL[:, :, 0, :]).reshape(b * (s // 128), H)
    cumf = cum.reshape(b * s, H)
    dtmv = dtp * 0.125

    in_maps = []
    for i in range(NCORES):
        h0 = i * c.NH
        ch0 = i * c.CL
        rows = np.concatenate([
            np.arange(ch0, ch0 + c.CL),             # gate
            np.arange(I + ch0, I + ch0 + c.CL),     # x
            np.arange(2 * I, 2 * I + 2 * N),        # B, C
        ])
        wiv = np.ascontiguousarray(
            np.asarray(W_in, np.float32)[rows, :].T).astype(bf16)
        wov = np.ascontiguousarray(
            np.asarray(W_out, np.float32)[:, ch0:ch0 + c.CL].T).astype(bf16)
        crows = np.concatenate([
            np.arange(ch0, ch0 + c.CL),
            np.arange(I, I + 2 * N),
        ])
        cwv = np.ascontiguousarray(np.asarray(conv_weight, np.float32)[crows, :])
        cbv = np.ascontiguousarray(
            np.asarray(conv_bias, np.float32)[crows, None])
        drv = np.repeat(np.asarray(D, np.float32)[h0:h0 + c.NH] * 0.5, P)[None, :]
        nwv = (np.asarray(norm_weight, np.float32)[ch0:ch0 + c.CL] * 0.5)[None, :]
        hsl = slice(h0, h0 + c.NH)
        in_maps.append({
            "hT": hTv,
            "wi": wiv,
            "wo": wov,
            "cw": cwv,
            "cbh": cbv,
            "drow": np.ascontiguousarray(drv),
            "nwrow": np.ascontiguousarray(nwv),
            "cumT": np.ascontiguousarray(cumf[:, hsl]),
            "cumR": np.ascontiguousarray(cumf[:, hsl].T),
            "e8": np.ascontiguousarray(e8v.reshape(b * s, H)[:, hsl]),
            "v8": np.ascontiguousarray(v8v.reshape(b * s, H)[:, hsl]),
            "dtm": np.ascontiguousarray(dtmv[:, hsl]),
            "wl": np.ascontiguousarray(wlv[:, hsl]).reshape(1, -1),
        })
    return in_maps


def host_gather(results, b=2, s=2048, cfg=None):
    c = cfg or Cfg()
    pout = np.zeros((b * s, HID), np.float64)
    ssqs = np.zeros((b * s,), np.float64)
    for r in results:
        pout += r["pout"].astype(np.float64)
        ssqs += r["ssq"][:, 0].astype(np.float64) * 0.25
    var = ssqs / I
    rr = 1.0 / np.sqrt(var + EPS)
    out = (pout * rr[:, None]).astype(np.float32).reshape(b, s, HID)

    cs = np.zeros((b, CONV_DIM, KCONV - 1), np.float32)
    for i, r in enumerate(results):
        cs[:, i * c.CL:(i + 1) * c.CL, :] = r["cstate"][:, :c.CL, :]
    cs[:, I:, :] = results[0]["cstate"][:, c.CL:, :]

    hs = np.zeros((b, H, P, N), np.float32)
    for i, r in enumerate(results):
        # hstate [b, N, CL] -> [b, nh, P, N]
        hv = r["hstate"].reshape(b, N, c.NH, P).transpose(0, 2, 3, 1)
        hs[:, i * c.NH:(i + 1) * c.NH] = hv
    return out, cs, hs


_cache = {}
TRACE = False
LAST = {}


def _install_ntff_hook():
    import sys as _sys
    import types as _types
    if 'antenv.axon_hooks' in _sys.modules:
        return
    try:
        import antenv
        mod = _types.ModuleType('antenv.axon_hooks')
        state = {'hook': None}
        mod.set_axon_ntff_profile_hook = lambda h: state.__setitem__('hook', h)
        mod.get_axon_ntff_profile_hook = lambda: state['hook']
        _sys.modules['antenv.axon_hooks'] = mod
        antenv.axon_hooks = mod
        from trn_agent_boot.trn_boot import _ntff_profile_via_ctypes
        mod.set_axon_ntff_profile_hook(
            _ntff_profile_via_ctypes('/opt/axon/libaxon_pjrt.so'))
    except Exception:
        pass


def kernel(hidden_states, W_in, conv_weight, conv_bias, dt_bias, A_log, D,
           norm_weight, W_out):
    from concourse import bass_utils
    cfg = Cfg()
    if "nc" not in _cache:
        _cache["nc"] = build_nc(cfg)
    nc = _cache["nc"]
    in_maps = host_prep(hidden_states, W_in, conv_weight, conv_bias, dt_bias,
                        A_log, D, norm_weight, W_out, cfg)
    if TRACE:
        _install_ntff_hook()
    res = bass_utils.run_bass_kernel_spmd(nc, in_maps,
                                          core_ids=list(range(NCORES)),
                                          trace=TRACE)
    LAST["exec_time_ns"] = res.exec_time_ns
    LAST["results"] = res
    b, s, _ = hidden_states.shape
    return host_gather(res.results, b, s, cfg)
